# revision 30
# baseline (speedup 1.0000x reference)
"""MLA (multi-head latent attention) Trainium2 kernel, SPMD over 8 NeuronCores.

Sharding: core c = 4*b + 2*jh + jq handles batch b.
  - Queries: stride-2 interleave by jq — core owns queries {jq, jq+2, ...}
    (1024 local). Causal structure is identical on every core: key chunk kc
    (keys 128kc..128kc+127) is visible only to local columns >= 64kc, and the
    partial band is always the first 64 columns with a fixed [128,64] mask.
  - Heads: jh selects heads 8jh..8jh+7 (4 groups of 2). The out-projection
    contracts only the local 8 heads; the two jh partials per (b, jq) are
    summed on the host during unshard (along with the bias).
  - kv latents / k_rope are built for the full batch on every core (cheap
    collectives are not available at useful bandwidth here).

All on-chip tensors use transposed ([feature, token]) layouts so every matmul
contracts over the partition dim with no on-chip transposes. rotate_half is
folded into host-permuted weight copies; 1/sqrt(dh) into the q weights;
softmax skips the max-pass (scores bounded). Row-sums come from ones-matmul
accumulation chains in PSUM (no DVE tree). Scores/exp/ctx/row-sums run on
ragged suffix-aligned chains covering only the visible ~53% region.

Scheduling notes: scalar does only exp in the attention phase (copies live on
vector); causal band masks on gpsimd; k_rope matmuls for two token tiles run
concurrently via PE column-group tiling; out-projection interleaves the four
output-column chains per stationary ctx chunk and streams results straight
from PSUM to HBM; Wo is prefetched during the last attention group; x-tile
loads alternate between the sync and gpsimd DMA queues, are issued a full
token-pair ahead of their matmuls, and fat weight loads are deferred off the
startup critical path.
"""

import os
import sys
import types

for _p in ("/opt/trn_rl_repo", "/root/.axon_site/_ro/trn_rl_repo"):
    if os.path.isdir(_p) and _p not in sys.path:
        sys.path.append(_p)

import numpy as np
import ml_dtypes

import concourse.bass as bass
import concourse.bacc as bacc_mod
import concourse.mybir as mybir
from concourse.tile import TileContext
from concourse.vector_clock import ScopedClock
from concourse.bass_utils import run_bass_kernel_spmd

F32 = mybir.dt.float32
BF16 = mybir.dt.bfloat16
BF16NP = ml_dtypes.bfloat16

HID, H, LAT, R = 2048, 16, 512, 32
DH, C = 128, 96
B, S = 2, 2048
SQ = 1024         # queries per core (interleaved stride 2)
NKC = S // 128    # 16 key chunks of 128
NG, GH = 4, 2     # per-core: 4 head-groups of 2 heads (8 heads)
HL = NG * GH      # 8 local heads
EXP_T = mybir.ActivationFunctionType.Exp


def _patch_tile_drain():
    """The staged walrus rejects a Drain carrying >1 sync-wait. Move the
    TileContext tail-drain waits onto single-wait SP nops."""

    def _drain_and_barrier(self, tick_clock, wait_clock):
        drain_inst = self.nc.sync.drain()
        wait_clock.add_sem_waits(
            drain_inst.ins, ScopedClock({None: tick_clock.global_clock})
        )
        si = drain_inst.ins.sync_info
        if si is not None and len(si.on_wait) > 1:
            waits = list(si.on_wait)
            drain_inst.ins.sync_info = mybir.SyncInfo(
                on_wait=[], on_update=list(si.on_update)
            )
            for w in waits:
                nop = self.nc.sync.nop(nofuse=True)
                nop.ins.sync_info = mybir.SyncInfo(on_wait=[w], on_update=[])
        self.nc.all_engine_barrier()
        assert self.sems is not None
        popped = self.nc._tile_sem_poison_stack.pop()
        assert popped is self._sem_poison
        self.nc.clear_and_free_semaphores(list(self.sems.allocated().values()))
        self.nc.all_engine_barrier()

    TileContext._drain_and_barrier = _drain_and_barrier


def _install_ntff_hook():
    """antenv.axon_hooks is absent in this image; inject it and register the
    ctypes NTFF hook so trace=True / BASS_TRACE can profile."""
    try:
        import antenv

        if "antenv.axon_hooks" not in sys.modules:
            mod = types.ModuleType("antenv.axon_hooks")
            mod._hook = None

            def set_axon_ntff_profile_hook(h):
                mod._hook = h

            def get_axon_ntff_profile_hook():
                return mod._hook

            mod.set_axon_ntff_profile_hook = set_axon_ntff_profile_hook
            mod.get_axon_ntff_profile_hook = get_axon_ntff_profile_hook
            sys.modules["antenv.axon_hooks"] = mod
            antenv.axon_hooks = mod
        boot_dir = "/root/.axon_site/trn_agent_boot"
        so_path = "/opt/axon/libaxon_pjrt.so"
        if os.path.isdir(boot_dir) and os.path.exists(so_path):
            if boot_dir not in sys.path:
                sys.path.append(boot_dir)
            from trn_boot import _ntff_profile_via_ctypes

            hook = _ntff_profile_via_ctypes(so_path)
            if hook is not None:
                sys.modules["antenv.axon_hooks"].set_axon_ntff_profile_hook(hook)
    except Exception:
        pass


_patch_tile_drain()
_install_ntff_hook()


def _dram(nc, name, shape, dtype=F32, out=False):
    return nc.declare_dram_parameter(name, list(shape), dtype, isOutput=out)


def build_nc():
    nc = bacc_mod.Bacc("TRN2", num_devices=8)

    xbT = _dram(nc, "xbT", [128, 8, 2, S], BF16)          # x[b].T, hid-split
    xqT = _dram(nc, "xqT", [128, 8, 2, SQ], BF16)         # interleaved queries
    wdkv = _dram(nc, "wdkv", [128, 8, 2, LAT], BF16)
    wdq = _dram(nc, "wdq", [128, 8, 2, LAT], BF16)
    wkr2 = _dram(nc, "wkr2", [128, 8, 2, 2 * R], BF16)    # [rope | rot] cols
    wk = _dram(nc, "wk", [128, 4, HL, C], BF16)           # local heads only
    wv = _dram(nc, "wv", [128, 4, HL, DH], BF16)
    wqc = _dram(nc, "wqc", [128, NG, 4, GH, C], BF16)     # * 1/sqrt(DH)
    wqr = _dram(nc, "wqr", [128, NG, 4, 4 * R], BF16)     # [ro0|ro1|rot0|rot1]*s
    wo = _dram(nc, "wo", [128, HL, HID], BF16)
    cs4q_d = _dram(nc, "cs4q", [128, SQ])                 # [cos;cos;sin;sin] f32
    cs_k_d = _dram(nc, "cs_k", [2 * R, S], BF16)          # [cos;sin] keys
    maskb_d = _dram(nc, "maskb", [128, 64], BF16)
    out_d = _dram(nc, "out", [SQ, HID], out=True)

    with TileContext(nc) as tc:
        with tc.tile_pool(name="pers", bufs=1) as PERS:
            ctxT = PERS.tile([128, HL, SQ], BF16, tag="ctxT", name="ctxT")
            wo_s = PERS.tile([128, HL, HID], BF16, tag="wo", name="wo_s")
            maskb = PERS.tile([128, 64], BF16, tag="maskb", name="maskb")
            onesb = PERS.tile([128, 128], BF16, tag="ones", name="ones")
            pass  # maskb loaded later on the scalar queue
            nc.gpsimd.memset(onesb[:], 1.0)

            with tc.tile_pool(name="lat", bufs=1) as LATP:
                kv_latT = LATP.tile([128, 4, S], BF16, tag="kvlat",
                                    name="kvlat")
                q_latT = LATP.tile([128, 4, SQ], BF16, tag="qlat", name="qlat")
                krT = LATP.tile([R, S], BF16, tag="krT", name="krT")
                cs4q = LATP.tile([128, SQ], F32, tag="cs4q", name="cs4q")

                # ---- Phase 1: full kv latents + roped k_rope + q latents ----
                with tc.tile_pool(name="p1", bufs=1) as P1, \
                     tc.tile_pool(name="xs", bufs=6) as XS, \
                     tc.tile_pool(name="t1", bufs=2) as T1, \
                     tc.tile_pool(name="ps1", bufs=8, space="PSUM") as PS1:
                    wdkv_s = P1.tile([128, 8, 2, LAT], BF16, tag="wdkv",
                                     name="wdkv")
                    wdq_s = P1.tile([128, 8, 2, LAT], BF16, tag="wdq",
                                    name="wdq")
                    wkr2_s = P1.tile([128, 8, 2, 2 * R], BF16, tag="wkr2",
                                     name="wkr2")
                    csk2 = P1.tile([2 * R, S], BF16, tag="csk2", name="csk2")

                    # split weight loads so the first matmuls start early
                    nc.scalar.dma_start(wdkv_s[:, 0:1, :, :],
                                        wdkv[:, 0:1, :, :])
                    nc.scalar.dma_start(wkr2_s[:], wkr2[:, :, :, :])
                    nc.scalar.dma_start(wdkv_s[:, 1:8, :, :],
                                        wdkv[:, 1:8, :, :])
                    nc.scalar.dma_start(csk2[:], cs_k_d[:, :])

                    for tp in range(2):
                        t0, t1 = 2 * tp, 2 * tp + 1
                        sl0 = slice(t0 * 512, (t0 + 1) * 512)
                        sl1 = slice(t1 * 512, (t1 + 1) * 512)
                        xb0s, xb1s = [], []
                        # issue the whole pair's tile loads up front so the
                        # DMA queues run ahead of the matmul consumption
                        for hch in range(8):
                            xbt = XS.tile([128, 2, 512], BF16, tag="xb",
                                          name="xb", bufs=8)
                            xbu = XS.tile([128, 2, 512], BF16, tag="xbu",
                                          name="xbu", bufs=10)
                            xb0s.append(xbt)
                            xb1s.append(xbu)
                            eng = nc.sync if hch % 2 == 0 else nc.gpsimd
                            eng.dma_start(xbt[:], xbT[:, hch, :, sl0])
                            eng2 = nc.gpsimd if hch % 2 == 0 else nc.sync
                            eng2.dma_start(xbu[:], xbT[:, hch, :, sl1])
                        # pass A: kv chains for t0; kr for BOTH tiles packed
                        # into one PSUM bank via column-group tiling
                        pss = [PS1.tile([128, 512], F32, tag="l",
                                        name=f"l{lc}") for lc in range(4)]
                        pkr2 = PS1.tile([128, 512], F32, tag="l", name="pkr2")
                        for hch in range(8):
                            xbt = xb0s[hch]
                            xbu = xb1s[hch]
                            for two in range(2):
                                st = (hch == 0 and two == 0)
                                sp = (hch == 7 and two == 1)
                                for lc in range(4):
                                    nc.tensor.matmul(
                                        pss[lc][:],
                                        lhsT=wdkv_s[:, hch, two,
                                                    lc * 128:(lc + 1) * 128],
                                        rhs=xbt[:, two, :],
                                        start=st, stop=sp,
                                    )
                                nc.tensor.matmul(
                                    pkr2[0:2 * R, :],
                                    lhsT=wkr2_s[:, hch, two, :],
                                    rhs=xbt[:, two, :],
                                    start=st, stop=sp,
                                    tile_position=(0, 0),
                                )
                                nc.tensor.matmul(
                                    pkr2[2 * R:4 * R, :],
                                    lhsT=wkr2_s[:, hch, two, :],
                                    rhs=xbu[:, two, :],
                                    start=st, stop=sp,
                                    tile_position=(0, 2 * R),
                                )
                        for lc in range(4):
                            nc.scalar.copy(kv_latT[:, lc, sl0], pss[lc][:])
                        for ti, tsl in ((0, sl0), (1, sl1)):
                            t2a = T1.tile([R, 512], F32, tag="t2a", name="t2a")
                            t2b = T1.tile([R, 512], F32, tag="t2b", name="t2b")
                            o = 2 * R * ti
                            nc.vector.tensor_mul(t2a[:], pkr2[o:o + R, :],
                                                 csk2[0:R, tsl])
                            nc.vector.tensor_mul(t2b[:],
                                                 pkr2[o + R:o + 2 * R, :],
                                                 csk2[R:2 * R, tsl])
                            nc.gpsimd.tensor_add(krT[:, tsl], t2a[:], t2b[:])
                        if tp == 0:
                            nc.scalar.dma_start(wdq_s[:], wdq[:, :, :, :])
                        # pass B: kv chains for t1 (tiles already resident)
                        pst = [PS1.tile([128, 512], F32, tag="l",
                                        name=f"m{lc}") for lc in range(4)]
                        for hch in range(8):
                            xbu = xb1s[hch]
                            for two in range(2):
                                st = (hch == 0 and two == 0)
                                sp = (hch == 7 and two == 1)
                                for lc in range(4):
                                    nc.tensor.matmul(
                                        pst[lc][:],
                                        lhsT=wdkv_s[:, hch, two,
                                                    lc * 128:(lc + 1) * 128],
                                        rhs=xbu[:, two, :],
                                        start=st, stop=sp,
                                    )
                        for lc in range(4):
                            nc.scalar.copy(kv_latT[:, lc, sl1], pst[lc][:])
                        if tp == 1:
                            nc.scalar.dma_start(cs4q[:], cs4q_d[:, :])
                            nc.scalar.dma_start(maskb[:], maskb_d[:, :])

                    # q latents over the core's own (interleaved) queries
                    for qh in range(2):
                        qsl = slice(qh * 512, (qh + 1) * 512)
                        psq = [PS1.tile([128, 512], F32, tag="l",
                                        name=f"q{lc}") for lc in range(4)]
                        for hch in range(8):
                            xqt = XS.tile([128, 2, 512], BF16, tag="xb",
                                          name="xq", bufs=8)
                            eng = nc.sync if hch % 2 == 0 else nc.gpsimd
                            eng.dma_start(xqt[:], xqT[:, hch, :, qsl])
                            for two in range(2):
                                st = (hch == 0 and two == 0)
                                sp = (hch == 7 and two == 1)
                                for lc in range(4):
                                    nc.tensor.matmul(
                                        psq[lc][:],
                                        lhsT=wdq_s[:, hch, two,
                                                   lc * 128:(lc + 1) * 128],
                                        rhs=xqt[:, two, :],
                                        start=st, stop=sp,
                                    )
                        for lc in range(4):
                            nc.scalar.copy(q_latT[:, lc, qsl], psq[lc][:])

                # ---------- Phase 2a: builds + attention ----------
                with tc.tile_pool(name="gw", bufs=2) as GW, \
                     tc.tile_pool(name="wvp", bufs=2) as WVP, \
                     tc.tile_pool(name="grp", bufs=2) as GRP, \
                     tc.tile_pool(name="vp", bufs=2) as VP, \
                     tc.tile_pool(name="et", bufs=2) as ET, \
                     tc.tile_pool(name="t4", bufs=2) as T4, \
                     tc.tile_pool(name="rcp", bufs=2) as RC, \
                     tc.tile_pool(name="ps_p", bufs=3, space="PSUM") as PP, \
                     tc.tile_pool(name="ps_s", bufs=2, space="PSUM") as PA, \
                     tc.tile_pool(name="ps_c", bufs=2, space="PSUM") as PSC, \
                     tc.tile_pool(name="ps_r", bufs=1, space="PSUM") as PSR:

                    kT_tiles = [None] * NG
                    qT_tiles = [None] * NG
                    v_tiles = [None] * 2
                    wv_tiles = [None] * 2
                    wk_tiles = [None] * NG

                    def build_kT_start(g):
                        wk_g = GW.tile([128, 4, GH * C], BF16, tag="wk",
                                       name=f"wk{g}")
                        wk_tiles[g] = wk_g
                        nc.sync.dma_start(wk_g[:],
                                          wk[:, :, GH * g:GH * (g + 1), :])
                        kT_g = GRP.tile([128, GH, S], BF16, tag="kT",
                                        name=f"kT{g}")
                        kT_tiles[g] = kT_g
                        for hh in range(GH):
                            nc.sync.dma_start(kT_g[C:128, hh, :], krT[:, :])

                    def build_kT_head(g, hh):
                        wk_g, kT_g = wk_tiles[g], kT_tiles[g]
                        for tq in range(4):
                            pk = PP.tile([128, 512], F32, tag="p", name="pk")
                            for lc in range(4):
                                nc.tensor.matmul(
                                    pk[0:C, :],
                                    lhsT=wk_g[:, lc, hh * C:(hh + 1) * C],
                                    rhs=kv_latT[:, lc, tq * 512:(tq + 1) * 512],
                                    start=(lc == 0), stop=(lc == 3),
                                )
                            nc.vector.tensor_copy(
                                kT_g[0:C, hh, tq * 512:(tq + 1) * 512],
                                pk[0:C, :],
                            )

                    def build_kT(g):
                        build_kT_start(g)
                        build_kT_head(g, 0)
                        build_kT_head(g, 1)

                    def build_v_half(vt, half):
                        """v for head-quad vt (4 heads), token half `half`."""
                        if half == 0:
                            wv_p = WVP.tile([128, 4, 4 * DH], BF16, tag="wv",
                                            name=f"wv{vt}")
                            wv_tiles[vt] = wv_p
                            nc.sync.dma_start(
                                wv_p[:], wv[:, :, 4 * vt:4 * (vt + 1), :]
                            )
                            v_p = VP.tile([128, NKC, 512], BF16, tag="v",
                                          name=f"v{vt}")
                            v_tiles[vt] = v_p
                        wv_p = wv_tiles[vt]
                        v_p = v_tiles[vt]
                        for kc in range(8 * half, 8 * half + 8):
                            pv = PP.tile([128, 512], F32, tag="p", name="pv")
                            for lc in range(4):
                                nc.tensor.matmul(
                                    pv[:],
                                    lhsT=kv_latT[:, lc, kc * 128:(kc + 1) * 128],
                                    rhs=wv_p[:, lc, :],
                                    start=(lc == 0), stop=(lc == 3),
                                )
                            nc.vector.tensor_copy(v_p[:, kc, :], pv[:])

                    def q_proj(g):
                        """q content + roped rope rows -> qT_tiles[g]."""
                        wq_g = GW.tile([128, 4, GH, C], BF16, tag="wq",
                                       name=f"wq{g}")
                        wqr_g = GW.tile([128, 4, 4 * R], BF16, tag="wqr",
                                        name=f"wqr{g}")
                        nc.sync.dma_start(wq_g[:], wqc[:, g, :, :, :])
                        nc.sync.dma_start(wqr_g[:], wqr[:, g, :, :])
                        qT_g = GRP.tile([128, GH, SQ], BF16, tag="qT",
                                        name=f"qT{g}")
                        qT_tiles[g] = qT_g
                        for hh in range(GH):
                            for qh in range(2):
                                qsl = slice(qh * 512, (qh + 1) * 512)
                                pqc = PP.tile([128, 512], F32, tag="p",
                                              name="pqc")
                                for lc in range(4):
                                    nc.tensor.matmul(
                                        pqc[0:C, :],
                                        lhsT=wq_g[:, lc, hh, :],
                                        rhs=q_latT[:, lc, qsl],
                                        start=(lc == 0), stop=(lc == 3),
                                    )
                                nc.vector.tensor_copy(qT_g[0:C, hh, qsl],
                                                      pqc[0:C, :])
                        for qh in range(2):
                            qsl = slice(qh * 512, (qh + 1) * 512)
                            psr = PP.tile([128, 512], F32, tag="p",
                                          name="psr")
                            for lc in range(4):
                                nc.tensor.matmul(
                                    psr[:],
                                    lhsT=wqr_g[:, lc, :],
                                    rhs=q_latT[:, lc, qsl],
                                    start=(lc == 0), stop=(lc == 3),
                                )
                            ta = T4.tile([2 * R, 512], F32, tag="ta",
                                         name="ta")
                            tb = T4.tile([2 * R, 512], F32, tag="tb",
                                         name="tb")
                            t64 = T4.tile([2 * R, 512], BF16, tag="t64",
                                          name="t64")
                            nc.vector.tensor_mul(ta[:], psr[0:2 * R, :],
                                                 cs4q[0:2 * R, qsl])
                            nc.vector.tensor_mul(tb[:], psr[2 * R:4 * R, :],
                                                 cs4q[2 * R:4 * R, qsl])
                            nc.gpsimd.tensor_add(t64[:], ta[:], tb[:])
                            for hh in range(GH):
                                nc.sync.dma_start(
                                    qT_g[C:128, hh, qsl],
                                    t64[R * hh:R * (hh + 1), :],
                                )

                    def attn_scores(g, hh, half, kcs):
                        """scores+exp+mask chunks; returns list of et tiles."""
                        kT_g, qT_g = kT_tiles[g], qT_tiles[g]
                        ets = []
                        for kc in kcs:
                            if half == 0:
                                w = 512 - 64 * kc
                                q0 = 64 * kc
                                banded = True
                            else:
                                w = min(512, SQ - 64 * kc)
                                q0 = max(512, 64 * kc)
                                banded = (kc >= 8)
                            ps = PA.tile([128, w], F32, tag="s",
                                         name=f"s{half}_{kc}")
                            nc.tensor.matmul(
                                ps[:],
                                lhsT=kT_g[:, hh, kc * 128:(kc + 1) * 128],
                                rhs=qT_g[:, hh, q0:q0 + w],
                                start=True, stop=True,
                            )
                            et = ET.tile([128, w], BF16,
                                         tag=f"e{half}_{kc}",
                                         name=f"e{half}_{kc}")
                            nc.scalar.activation(et[:], ps[:], EXP_T)
                            if banded:
                                nc.gpsimd.tensor_mul(et[:, 0:64], et[:, 0:64],
                                                     maskb[:])
                            ets.append(et)
                        return ets

                    def attn_ctx(g, hh, half, ets):
                        """ctx chain + row-sum chain + normalize -> ctxT."""
                        h = GH * g + hh
                        v_p = v_tiles[h // 4]
                        hp = h % 4
                        qsl = slice(half * 512, (half + 1) * 512)
                        kcs = range(8) if half == 0 else range(NKC)
                        last = kcs[-1]
                        pctx = PSC.tile([128, 512], F32, tag="c", name="pc")
                        for kc in kcs:
                            lo = max(0, 64 * kc - half * 512)
                            nc.tensor.matmul(
                                pctx[:, lo:512],
                                lhsT=v_p[:, kc, hp * 128:(hp + 1) * 128],
                                rhs=ets[kc][:],
                                start=(kc == 0), stop=(kc == last),
                            )
                        prs = PSR.tile([128, 512], F32, tag="r", name="pr")
                        for kc in kcs:
                            lo = max(0, 64 * kc - half * 512)
                            nc.tensor.matmul(
                                prs[:, lo:512],
                                lhsT=onesb[:],
                                rhs=ets[kc][:],
                                start=(kc == 0), stop=(kc == last),
                            )
                        rc = RC.tile([128, 512], F32, tag="rc", name="rc")
                        nc.vector.reciprocal_approx_fast(out=rc[:], in_=prs[:])
                        nc.vector.tensor_mul(ctxT[:, h, qsl], pctx[:], rc[:])

                    # software-pipelined emission: weave next group's builds
                    # between score half-batches so the tensor engine has
                    # filler while exp drains the score ring
                    build_kT(0)
                    build_v_half(0, 0)
                    build_v_half(0, 1)
                    q_proj(0)
                    for g in range(NG):
                        if g == NG - 1:
                            for oc in range(4):
                                nc.sync.dma_start(
                                    wo_s[:, :, oc * 512:(oc + 1) * 512],
                                    wo[:, :, oc * 512:(oc + 1) * 512],
                                )
                        if g + 1 < NG:
                            build_kT_start(g + 1)
                        e00 = attn_scores(g, 0, 0, range(8))
                        if g + 1 < NG:
                            build_kT_head(g + 1, 0)
                        e01 = attn_scores(g, 0, 1, range(NKC))
                        attn_ctx(g, 0, 0, e00)
                        if g + 1 < NG:
                            build_kT_head(g + 1, 1)
                        e10 = attn_scores(g, 1, 0, range(8))
                        attn_ctx(g, 0, 1, e01)
                        if g == 0:
                            build_v_half(1, 0)
                        if g == 1:
                            build_v_half(1, 1)
                        e11 = attn_scores(g, 1, 1, range(NKC))
                        if g + 1 < NG:
                            q_proj(g + 1)
                        attn_ctx(g, 1, 0, e10)
                        attn_ctx(g, 1, 1, e11)

            # ---------- Phase 2b: out-projection ----------
            with tc.tile_pool(name="st", bufs=6) as STP, \
                 tc.tile_pool(name="ps_o", bufs=8, space="PSUM") as PO:
                for tq in range(8):
                    pos = [PO.tile([128, 512], F32, tag="o", name=f"po{oc}")
                           for oc in range(4)]
                    for h in range(HL):
                        for oc in range(4):
                            nc.tensor.matmul(
                                pos[oc][:],
                                lhsT=ctxT[:, h, tq * 128:(tq + 1) * 128],
                                rhs=wo_s[:, h, oc * 512:(oc + 1) * 512],
                                start=(h == 0), stop=(h == HL - 1),
                            )
                    for oc in range(4):
                        osl = slice(oc * 512, (oc + 1) * 512)
                        st = STP.tile([128, 512], F32, tag="st", name="st")
                        if oc % 2 == 0:
                            nc.scalar.copy(st[:], pos[oc][:])
                        else:
                            nc.vector.tensor_copy(st[:], pos[oc][:])
                        deng = (nc.sync, nc.gpsimd, nc.scalar)[oc % 3]
                        deng.dma_start(
                            out_d[tq * 128:(tq + 1) * 128, osl], st[:]
                        )

    nc.compile()
    return nc


def _rot_rows(w):
    # rows of w are the rope dim; rot(w) @ lat == rotate_half(w @ lat)
    hR = w.shape[0] // 2
    return np.concatenate([-w[hR:], w[:hR]], axis=0)


def _hidsplit(m):
    """[HID, cols] -> [128, 8, 2, cols] with hid row (2*hch+two)*128+p."""
    cols = m.shape[1]
    return np.ascontiguousarray(
        m.reshape(16, 128, cols).transpose(1, 0, 2).reshape(128, 8, 2, cols)
    )


def _latsplit(m):
    """[LAT, cols] -> [128, 4, cols] with lat row lc*128+p."""
    cols = m.shape[1]
    return np.ascontiguousarray(
        m.reshape(4, 128, cols).transpose(1, 0, 2)
    )


def _prep_inputs(inputs):
    x = np.asarray(inputs["x"], np.float32)
    Wq_down = np.asarray(inputs["Wq_down"], np.float32)
    Wq_up = np.asarray(inputs["Wq_up"], np.float32)
    Wq_rope = np.asarray(inputs["Wq_rope"], np.float32)
    Wkv_down = np.asarray(inputs["Wkv_down"], np.float32)
    Wk_up = np.asarray(inputs["Wk_up"], np.float32)
    Wk_rope = np.asarray(inputs["Wk_rope"], np.float32)
    Wv_up = np.asarray(inputs["Wv_up"], np.float32)
    Wo = np.asarray(inputs["Wo"], np.float32)

    s = np.float32(1.0 / np.sqrt(DH))

    wdkv_h = _hidsplit(Wkv_down.T).astype(BF16NP)
    wdq_h = _hidsplit(Wq_down.T).astype(BF16NP)
    wkr2_h = _hidsplit(
        np.concatenate([Wk_rope.T, _rot_rows(Wk_rope).T], axis=1)
    ).astype(BF16NP)
    wk_full = _latsplit(Wk_up.T).reshape(128, 4, H, C)
    wv_full = _latsplit(Wv_up.T).reshape(128, 4, H, DH)
    wqc_full = _latsplit((Wq_up * s).T).reshape(128, 4, H, C)
    wo_full = np.ascontiguousarray(
        Wo.T.reshape(H, 128, HID).transpose(1, 0, 2)
    )

    inv_freq = (1.0 / (10000.0 ** (np.arange(0, R, 2, dtype=np.float32) / R)))
    t = np.arange(S, dtype=np.float32)
    freqs = t[:, None] * inv_freq[None, :]
    emb = np.concatenate([freqs, freqs], axis=-1)          # [S, R]
    cos = np.cos(emb).astype(np.float32)
    sin = np.sin(emb).astype(np.float32)

    cs_k_h = np.ascontiguousarray(
        np.concatenate([cos.T, sin.T], axis=0)).astype(BF16NP)
    xbT_h = [_hidsplit(x[b].T.astype(np.float32)).astype(BF16NP)
             for b in range(B)]

    # per-jh-sliced head weights
    wk_h, wv_h, wqc_h, wqr_h, wo_h = {}, {}, {}, {}, {}
    for jh in range(2):
        hs = slice(8 * jh, 8 * jh + 8)
        wk_h[jh] = np.ascontiguousarray(wk_full[:, :, hs, :]).astype(BF16NP)
        wv_h[jh] = np.ascontiguousarray(wv_full[:, :, hs, :]).astype(BF16NP)
        wqc_h[jh] = np.ascontiguousarray(
            wqc_full[:, :, hs, :].reshape(128, 4, NG, GH, C)
            .transpose(0, 2, 1, 3, 4)
        ).astype(BF16NP)
        # per group: [ro_h0 | ro_h1 | rot_h0 | rot_h1] * s
        wqr4 = np.empty((LAT, NG, 4 * R), np.float32)
        for g in range(NG):
            h0 = 8 * jh + GH * g
            q0 = Wq_rope[h0 * R:(h0 + 1) * R] * s
            q1 = Wq_rope[(h0 + 1) * R:(h0 + 2) * R] * s
            wqr4[:, g, 0 * R:1 * R] = q0.T
            wqr4[:, g, 1 * R:2 * R] = q1.T
            wqr4[:, g, 2 * R:3 * R] = _rot_rows(q0).T
            wqr4[:, g, 3 * R:4 * R] = _rot_rows(q1).T
        wqr_h[jh] = np.ascontiguousarray(
            _latsplit(wqr4.reshape(LAT, NG * 4 * R))
            .reshape(128, 4, NG, 4 * R).transpose(0, 2, 1, 3)
        ).astype(BF16NP)
        wo_h[jh] = np.ascontiguousarray(wo_full[:, hs, :]).astype(BF16NP)

    par = np.arange(128)[:, None]
    mar = np.arange(64)[None, :]

    in_maps = []
    for c in range(8):
        b, sub = divmod(c, 4)
        jh, jq = divmod(sub, 2)
        qi = jq + 2 * np.arange(SQ)
        cs4q = np.empty((128, SQ), np.float32)
        cs4q[0:R] = cos[qi].T
        cs4q[R:2 * R] = cos[qi].T
        cs4q[2 * R:3 * R] = sin[qi].T
        cs4q[3 * R:4 * R] = sin[qi].T
        maskb = np.where(par <= 2 * mar + jq, 1.0, 0.0)
        in_maps.append({
            "xbT": xbT_h[b],
            "xqT": _hidsplit(x[b, jq::2].T.astype(np.float32)).astype(BF16NP),
            "wdkv": wdkv_h, "wdq": wdq_h, "wkr2": wkr2_h,
            "wk": wk_h[jh], "wv": wv_h[jh], "wqc": wqc_h[jh],
            "wqr": wqr_h[jh], "wo": wo_h[jh],
            "cs4q": np.ascontiguousarray(cs4q),
            "cs_k": cs_k_h,
            "maskb": maskb.astype(BF16NP),
        })
    return in_maps


_NC_CACHE = None


def run_on_cores(inputs, trace=False):
    global _NC_CACHE
    if _NC_CACHE is None:
        _NC_CACHE = build_nc()
    nc = _NC_CACHE
    in_maps = _prep_inputs(inputs)
    res = run_bass_kernel_spmd(nc, in_maps, list(range(8)), trace=trace)
    out = np.zeros((B, S, HID), np.float32)
    for c in range(8):
        b, sub = divmod(c, 4)
        jh, jq = divmod(sub, 2)
        out[b, jq::2, :] += res.results[c]["out"]
    out += np.asarray(inputs["bo"], np.float32)[None, None, :]
    return out, res


def kernel(**inputs):
    out, _ = run_on_cores(inputs, trace=False)
    return out


# revision 31
# speedup vs baseline: 1.0031x; 1.0031x over previous
"""MLA (multi-head latent attention) Trainium2 kernel, SPMD over 8 NeuronCores.

Sharding: core c = 4*b + 2*jh + jq handles batch b.
  - Queries: stride-2 interleave by jq — core owns queries {jq, jq+2, ...}
    (1024 local). Causal structure is identical on every core: key chunk kc
    (keys 128kc..128kc+127) is visible only to local columns >= 64kc, and the
    partial band is always the first 64 columns with a fixed [128,64] mask.
  - Heads: jh selects heads 8jh..8jh+7 (4 groups of 2). The out-projection
    contracts only the local 8 heads; the two jh partials per (b, jq) are
    summed on the host during unshard (along with the bias).
  - kv latents / k_rope are built for the full batch on every core (cheap
    collectives are not available at useful bandwidth here).

All on-chip tensors use transposed ([feature, token]) layouts so every matmul
contracts over the partition dim with no on-chip transposes. rotate_half is
folded into host-permuted weight copies; 1/sqrt(dh) into the q weights;
softmax skips the max-pass (scores bounded). Row-sums come from ones-matmul
accumulation chains in PSUM (no DVE tree). Scores/exp/ctx/row-sums run on
ragged suffix-aligned chains covering only the visible ~53% region.

Scheduling notes: scalar does only exp in the attention phase (copies live on
vector); causal band masks on gpsimd; k_rope matmuls for two token tiles run
concurrently via PE column-group tiling; out-projection interleaves the four
output-column chains per stationary ctx chunk and streams results straight
from PSUM to HBM; Wo is prefetched during the last attention group; x-tile
loads alternate between the sync and gpsimd DMA queues, are issued a full
token-pair ahead of their matmuls, and fat weight loads are deferred off the
startup critical path.
"""

import os
import sys
import types

for _p in ("/opt/trn_rl_repo", "/root/.axon_site/_ro/trn_rl_repo"):
    if os.path.isdir(_p) and _p not in sys.path:
        sys.path.append(_p)

import numpy as np
import ml_dtypes

import concourse.bass as bass
import concourse.bacc as bacc_mod
import concourse.mybir as mybir
from concourse.tile import TileContext
from concourse.vector_clock import ScopedClock
from concourse.bass_utils import run_bass_kernel_spmd

F32 = mybir.dt.float32
BF16 = mybir.dt.bfloat16
BF16NP = ml_dtypes.bfloat16

HID, H, LAT, R = 2048, 16, 512, 32
DH, C = 128, 96
B, S = 2, 2048
SQ = 1024         # queries per core (interleaved stride 2)
NKC = S // 128    # 16 key chunks of 128
NG, GH = 4, 2     # per-core: 4 head-groups of 2 heads (8 heads)
HL = NG * GH      # 8 local heads
EXP_T = mybir.ActivationFunctionType.Exp


def _patch_tile_drain():
    """The staged walrus rejects a Drain carrying >1 sync-wait. Move the
    TileContext tail-drain waits onto single-wait SP nops."""

    def _drain_and_barrier(self, tick_clock, wait_clock):
        drain_inst = self.nc.sync.drain()
        wait_clock.add_sem_waits(
            drain_inst.ins, ScopedClock({None: tick_clock.global_clock})
        )
        si = drain_inst.ins.sync_info
        if si is not None and len(si.on_wait) > 1:
            waits = list(si.on_wait)
            drain_inst.ins.sync_info = mybir.SyncInfo(
                on_wait=[], on_update=list(si.on_update)
            )
            for w in waits:
                nop = self.nc.sync.nop(nofuse=True)
                nop.ins.sync_info = mybir.SyncInfo(on_wait=[w], on_update=[])
        self.nc.all_engine_barrier()
        assert self.sems is not None
        popped = self.nc._tile_sem_poison_stack.pop()
        assert popped is self._sem_poison
        self.nc.clear_and_free_semaphores(list(self.sems.allocated().values()))
        self.nc.all_engine_barrier()

    TileContext._drain_and_barrier = _drain_and_barrier


def _install_ntff_hook():
    """antenv.axon_hooks is absent in this image; inject it and register the
    ctypes NTFF hook so trace=True / BASS_TRACE can profile."""
    try:
        import antenv

        if "antenv.axon_hooks" not in sys.modules:
            mod = types.ModuleType("antenv.axon_hooks")
            mod._hook = None

            def set_axon_ntff_profile_hook(h):
                mod._hook = h

            def get_axon_ntff_profile_hook():
                return mod._hook

            mod.set_axon_ntff_profile_hook = set_axon_ntff_profile_hook
            mod.get_axon_ntff_profile_hook = get_axon_ntff_profile_hook
            sys.modules["antenv.axon_hooks"] = mod
            antenv.axon_hooks = mod
        boot_dir = "/root/.axon_site/trn_agent_boot"
        so_path = "/opt/axon/libaxon_pjrt.so"
        if os.path.isdir(boot_dir) and os.path.exists(so_path):
            if boot_dir not in sys.path:
                sys.path.append(boot_dir)
            from trn_boot import _ntff_profile_via_ctypes

            hook = _ntff_profile_via_ctypes(so_path)
            if hook is not None:
                sys.modules["antenv.axon_hooks"].set_axon_ntff_profile_hook(hook)
    except Exception:
        pass


_patch_tile_drain()
_install_ntff_hook()


def _dram(nc, name, shape, dtype=F32, out=False):
    return nc.declare_dram_parameter(name, list(shape), dtype, isOutput=out)


def build_nc():
    nc = bacc_mod.Bacc("TRN2", num_devices=8)

    xbT = _dram(nc, "xbT", [128, 8, 2, S], BF16)          # x[b].T, hid-split
    xqT = _dram(nc, "xqT", [128, 8, 2, SQ], BF16)         # interleaved queries
    wdkv = _dram(nc, "wdkv", [128, 8, 2, LAT], BF16)
    wdq = _dram(nc, "wdq", [128, 8, 2, LAT], BF16)
    wkr2 = _dram(nc, "wkr2", [128, 8, 2, 2 * R], BF16)    # [rope | rot] cols
    wk = _dram(nc, "wk", [128, 4, HL, C], BF16)           # local heads only
    wv = _dram(nc, "wv", [128, 4, HL, DH], BF16)
    wqc = _dram(nc, "wqc", [128, NG, 4, GH, C], BF16)     # * 1/sqrt(DH)
    wqr = _dram(nc, "wqr", [128, NG, 4, 4 * R], BF16)     # [ro0|ro1|rot0|rot1]*s
    wo = _dram(nc, "wo", [128, HL, HID], BF16)
    cs4q_d = _dram(nc, "cs4q", [128, SQ])                 # [cos;cos;sin;sin] f32
    cs_k_d = _dram(nc, "cs_k", [2 * R, S], BF16)          # [cos;sin] keys
    maskb_d = _dram(nc, "maskb", [128, 64], BF16)
    out_d = _dram(nc, "out", [SQ, HID], out=True)

    with TileContext(nc) as tc:
        with tc.tile_pool(name="pers", bufs=1) as PERS:
            ctxT = PERS.tile([128, HL, SQ], BF16, tag="ctxT", name="ctxT")
            wo_s = PERS.tile([128, HL, HID], BF16, tag="wo", name="wo_s")
            maskb = PERS.tile([128, 64], BF16, tag="maskb", name="maskb")
            onesb = PERS.tile([128, 128], BF16, tag="ones", name="ones")
            pass  # maskb loaded later on the scalar queue
            nc.gpsimd.memset(onesb[:], 1.0)

            with tc.tile_pool(name="lat", bufs=1) as LATP:
                kv_latT = LATP.tile([128, 4, S], BF16, tag="kvlat",
                                    name="kvlat")
                q_latT = LATP.tile([128, 4, SQ], BF16, tag="qlat", name="qlat")
                krT = LATP.tile([R, S], BF16, tag="krT", name="krT")
                cs4q = LATP.tile([128, SQ], F32, tag="cs4q", name="cs4q")

                # ---- Phase 1: full kv latents + roped k_rope + q latents ----
                with tc.tile_pool(name="p1", bufs=1) as P1, \
                     tc.tile_pool(name="xs", bufs=6) as XS, \
                     tc.tile_pool(name="t1", bufs=2) as T1, \
                     tc.tile_pool(name="ps1", bufs=8, space="PSUM") as PS1:
                    wdkv_s = P1.tile([128, 8, 2, LAT], BF16, tag="wdkv",
                                     name="wdkv")
                    wdq_s = P1.tile([128, 8, 2, LAT], BF16, tag="wdq",
                                    name="wdq")
                    wkr2_s = P1.tile([128, 8, 2, 2 * R], BF16, tag="wkr2",
                                     name="wkr2")
                    csk2 = P1.tile([2 * R, S], BF16, tag="csk2", name="csk2")

                    # split weight loads so the first matmuls start early
                    nc.scalar.dma_start(wdkv_s[:, 0:1, :, :],
                                        wdkv[:, 0:1, :, :])
                    nc.scalar.dma_start(wkr2_s[:], wkr2[:, :, :, :])
                    nc.scalar.dma_start(wdkv_s[:, 1:8, :, :],
                                        wdkv[:, 1:8, :, :])
                    nc.scalar.dma_start(csk2[:], cs_k_d[:, :])

                    for tp in range(2):
                        t0, t1 = 2 * tp, 2 * tp + 1
                        sl0 = slice(t0 * 512, (t0 + 1) * 512)
                        sl1 = slice(t1 * 512, (t1 + 1) * 512)
                        xb0s, xb1s = [], []
                        # issue the whole pair's tile loads up front so the
                        # DMA queues run ahead of the matmul consumption
                        for hch in range(8):
                            xbt = XS.tile([128, 2, 512], BF16, tag="xb",
                                          name="xb", bufs=8)
                            xbu = XS.tile([128, 2, 512], BF16, tag="xbu",
                                          name="xbu", bufs=10)
                            xb0s.append(xbt)
                            xb1s.append(xbu)
                            eng = nc.sync if hch % 2 == 0 else nc.gpsimd
                            eng.dma_start(xbt[:], xbT[:, hch, :, sl0])
                            eng2 = nc.gpsimd if hch % 2 == 0 else nc.sync
                            eng2.dma_start(xbu[:], xbT[:, hch, :, sl1])
                        # pass A: kv chains for t0; kr for BOTH tiles packed
                        # into one PSUM bank via column-group tiling
                        pss = [PS1.tile([128, 512], F32, tag="l",
                                        name=f"l{lc}") for lc in range(4)]
                        pkr2 = PS1.tile([128, 512], F32, tag="l", name="pkr2")
                        for hch in range(8):
                            xbt = xb0s[hch]
                            xbu = xb1s[hch]
                            for two in range(2):
                                st = (hch == 0 and two == 0)
                                sp = (hch == 7 and two == 1)
                                for lc in range(4):
                                    nc.tensor.matmul(
                                        pss[lc][:],
                                        lhsT=wdkv_s[:, hch, two,
                                                    lc * 128:(lc + 1) * 128],
                                        rhs=xbt[:, two, :],
                                        start=st, stop=sp,
                                    )
                                nc.tensor.matmul(
                                    pkr2[0:2 * R, :],
                                    lhsT=wkr2_s[:, hch, two, :],
                                    rhs=xbt[:, two, :],
                                    start=st, stop=sp,
                                    tile_position=(0, 0),
                                )
                                nc.tensor.matmul(
                                    pkr2[2 * R:4 * R, :],
                                    lhsT=wkr2_s[:, hch, two, :],
                                    rhs=xbu[:, two, :],
                                    start=st, stop=sp,
                                    tile_position=(0, 2 * R),
                                )
                        for lc in range(4):
                            nc.scalar.copy(kv_latT[:, lc, sl0], pss[lc][:])
                        for ti, tsl in ((0, sl0), (1, sl1)):
                            t2a = T1.tile([R, 512], F32, tag="t2a", name="t2a")
                            t2b = T1.tile([R, 512], F32, tag="t2b", name="t2b")
                            o = 2 * R * ti
                            nc.vector.tensor_mul(t2a[:], pkr2[o:o + R, :],
                                                 csk2[0:R, tsl])
                            nc.vector.tensor_mul(t2b[:],
                                                 pkr2[o + R:o + 2 * R, :],
                                                 csk2[R:2 * R, tsl])
                            nc.gpsimd.tensor_add(krT[:, tsl], t2a[:], t2b[:])
                        if tp == 0:
                            nc.scalar.dma_start(wdq_s[:], wdq[:, :, :, :])
                        # pass B: kv chains for t1 (tiles already resident)
                        pst = [PS1.tile([128, 512], F32, tag="l",
                                        name=f"m{lc}") for lc in range(4)]
                        for hch in range(8):
                            xbu = xb1s[hch]
                            for two in range(2):
                                st = (hch == 0 and two == 0)
                                sp = (hch == 7 and two == 1)
                                for lc in range(4):
                                    nc.tensor.matmul(
                                        pst[lc][:],
                                        lhsT=wdkv_s[:, hch, two,
                                                    lc * 128:(lc + 1) * 128],
                                        rhs=xbu[:, two, :],
                                        start=st, stop=sp,
                                    )
                        for lc in range(4):
                            nc.scalar.copy(kv_latT[:, lc, sl1], pst[lc][:])
                        if tp == 1:
                            nc.scalar.dma_start(cs4q[:], cs4q_d[:, :])
                            nc.scalar.dma_start(maskb[:], maskb_d[:, :])

                    # q latents over the core's own (interleaved) queries
                    for qh in range(2):
                        qsl = slice(qh * 512, (qh + 1) * 512)
                        psq = [PS1.tile([128, 512], F32, tag="l",
                                        name=f"q{lc}") for lc in range(4)]
                        for hch in range(8):
                            xqt = XS.tile([128, 2, 512], BF16, tag="xb",
                                          name="xq", bufs=8)
                            eng = nc.sync if hch % 2 == 0 else nc.gpsimd
                            eng.dma_start(xqt[:], xqT[:, hch, :, qsl])
                            for two in range(2):
                                st = (hch == 0 and two == 0)
                                sp = (hch == 7 and two == 1)
                                for lc in range(4):
                                    nc.tensor.matmul(
                                        psq[lc][:],
                                        lhsT=wdq_s[:, hch, two,
                                                   lc * 128:(lc + 1) * 128],
                                        rhs=xqt[:, two, :],
                                        start=st, stop=sp,
                                    )
                        for lc in range(4):
                            nc.scalar.copy(q_latT[:, lc, qsl], psq[lc][:])

                # ---------- Phase 2a: builds + attention ----------
                with tc.tile_pool(name="gw", bufs=2) as GW, \
                     tc.tile_pool(name="wvp", bufs=2) as WVP, \
                     tc.tile_pool(name="grp", bufs=2) as GRP, \
                     tc.tile_pool(name="vp", bufs=2) as VP, \
                     tc.tile_pool(name="et", bufs=2) as ET, \
                     tc.tile_pool(name="t4", bufs=2) as T4, \
                     tc.tile_pool(name="rcp", bufs=2) as RC, \
                     tc.tile_pool(name="ps_p", bufs=2, space="PSUM") as PP, \
                     tc.tile_pool(name="ps_s", bufs=3, space="PSUM") as PA, \
                     tc.tile_pool(name="ps_c", bufs=2, space="PSUM") as PSC, \
                     tc.tile_pool(name="ps_r", bufs=1, space="PSUM") as PSR:

                    kT_tiles = [None] * NG
                    qT_tiles = [None] * NG
                    v_tiles = [None] * 2
                    wv_tiles = [None] * 2
                    wk_tiles = [None] * NG

                    def build_kT_start(g):
                        wk_g = GW.tile([128, 4, GH * C], BF16, tag="wk",
                                       name=f"wk{g}")
                        wk_tiles[g] = wk_g
                        nc.sync.dma_start(wk_g[:],
                                          wk[:, :, GH * g:GH * (g + 1), :])
                        kT_g = GRP.tile([128, GH, S], BF16, tag="kT",
                                        name=f"kT{g}")
                        kT_tiles[g] = kT_g
                        for hh in range(GH):
                            nc.sync.dma_start(kT_g[C:128, hh, :], krT[:, :])

                    def build_kT_head(g, hh):
                        wk_g, kT_g = wk_tiles[g], kT_tiles[g]
                        for tq in range(4):
                            pk = PP.tile([128, 512], F32, tag="p", name="pk")
                            for lc in range(4):
                                nc.tensor.matmul(
                                    pk[0:C, :],
                                    lhsT=wk_g[:, lc, hh * C:(hh + 1) * C],
                                    rhs=kv_latT[:, lc, tq * 512:(tq + 1) * 512],
                                    start=(lc == 0), stop=(lc == 3),
                                )
                            nc.vector.tensor_copy(
                                kT_g[0:C, hh, tq * 512:(tq + 1) * 512],
                                pk[0:C, :],
                            )

                    def build_kT(g):
                        build_kT_start(g)
                        build_kT_head(g, 0)
                        build_kT_head(g, 1)

                    def build_v_half(vt, half):
                        """v for head-quad vt (4 heads), token half `half`."""
                        if half == 0:
                            wv_p = WVP.tile([128, 4, 4 * DH], BF16, tag="wv",
                                            name=f"wv{vt}")
                            wv_tiles[vt] = wv_p
                            nc.sync.dma_start(
                                wv_p[:], wv[:, :, 4 * vt:4 * (vt + 1), :]
                            )
                            v_p = VP.tile([128, NKC, 512], BF16, tag="v",
                                          name=f"v{vt}")
                            v_tiles[vt] = v_p
                        wv_p = wv_tiles[vt]
                        v_p = v_tiles[vt]
                        for kc in range(8 * half, 8 * half + 8):
                            pv = PP.tile([128, 512], F32, tag="p", name="pv")
                            for lc in range(4):
                                nc.tensor.matmul(
                                    pv[:],
                                    lhsT=kv_latT[:, lc, kc * 128:(kc + 1) * 128],
                                    rhs=wv_p[:, lc, :],
                                    start=(lc == 0), stop=(lc == 3),
                                )
                            nc.vector.tensor_copy(v_p[:, kc, :], pv[:])

                    def q_proj(g):
                        """q content + roped rope rows -> qT_tiles[g]."""
                        wq_g = GW.tile([128, 4, GH, C], BF16, tag="wq",
                                       name=f"wq{g}")
                        wqr_g = GW.tile([128, 4, 4 * R], BF16, tag="wqr",
                                        name=f"wqr{g}")
                        nc.sync.dma_start(wq_g[:], wqc[:, g, :, :, :])
                        nc.sync.dma_start(wqr_g[:], wqr[:, g, :, :])
                        qT_g = GRP.tile([128, GH, SQ], BF16, tag="qT",
                                        name=f"qT{g}")
                        qT_tiles[g] = qT_g
                        for hh in range(GH):
                            for qh in range(2):
                                qsl = slice(qh * 512, (qh + 1) * 512)
                                pqc = PP.tile([128, 512], F32, tag="p",
                                              name="pqc")
                                for lc in range(4):
                                    nc.tensor.matmul(
                                        pqc[0:C, :],
                                        lhsT=wq_g[:, lc, hh, :],
                                        rhs=q_latT[:, lc, qsl],
                                        start=(lc == 0), stop=(lc == 3),
                                    )
                                nc.vector.tensor_copy(qT_g[0:C, hh, qsl],
                                                      pqc[0:C, :])
                        for qh in range(2):
                            qsl = slice(qh * 512, (qh + 1) * 512)
                            psr = PP.tile([128, 512], F32, tag="p",
                                          name="psr")
                            for lc in range(4):
                                nc.tensor.matmul(
                                    psr[:],
                                    lhsT=wqr_g[:, lc, :],
                                    rhs=q_latT[:, lc, qsl],
                                    start=(lc == 0), stop=(lc == 3),
                                )
                            ta = T4.tile([2 * R, 512], F32, tag="ta",
                                         name="ta")
                            tb = T4.tile([2 * R, 512], F32, tag="tb",
                                         name="tb")
                            t64 = T4.tile([2 * R, 512], BF16, tag="t64",
                                          name="t64")
                            nc.vector.tensor_mul(ta[:], psr[0:2 * R, :],
                                                 cs4q[0:2 * R, qsl])
                            nc.vector.tensor_mul(tb[:], psr[2 * R:4 * R, :],
                                                 cs4q[2 * R:4 * R, qsl])
                            nc.gpsimd.tensor_add(t64[:], ta[:], tb[:])
                            for hh in range(GH):
                                nc.sync.dma_start(
                                    qT_g[C:128, hh, qsl],
                                    t64[R * hh:R * (hh + 1), :],
                                )

                    def attn_scores(g, hh, half, kcs):
                        """scores+exp+mask chunks; returns list of et tiles."""
                        kT_g, qT_g = kT_tiles[g], qT_tiles[g]
                        ets = []
                        for kc in kcs:
                            if half == 0:
                                w = 512 - 64 * kc
                                q0 = 64 * kc
                                banded = True
                            else:
                                w = min(512, SQ - 64 * kc)
                                q0 = max(512, 64 * kc)
                                banded = (kc >= 8)
                            ps = PA.tile([128, w], F32, tag="s",
                                         name=f"s{half}_{kc}")
                            nc.tensor.matmul(
                                ps[:],
                                lhsT=kT_g[:, hh, kc * 128:(kc + 1) * 128],
                                rhs=qT_g[:, hh, q0:q0 + w],
                                start=True, stop=True,
                            )
                            et = ET.tile([128, w], BF16,
                                         tag=f"e{half}_{kc}",
                                         name=f"e{half}_{kc}")
                            nc.scalar.activation(et[:], ps[:], EXP_T)
                            if banded:
                                nc.gpsimd.tensor_mul(et[:, 0:64], et[:, 0:64],
                                                     maskb[:])
                            ets.append(et)
                        return ets

                    def attn_ctx(g, hh, half, ets):
                        """ctx chain + row-sum chain + normalize -> ctxT."""
                        h = GH * g + hh
                        v_p = v_tiles[h // 4]
                        hp = h % 4
                        qsl = slice(half * 512, (half + 1) * 512)
                        kcs = range(8) if half == 0 else range(NKC)
                        last = kcs[-1]
                        pctx = PSC.tile([128, 512], F32, tag="c", name="pc")
                        for kc in kcs:
                            lo = max(0, 64 * kc - half * 512)
                            nc.tensor.matmul(
                                pctx[:, lo:512],
                                lhsT=v_p[:, kc, hp * 128:(hp + 1) * 128],
                                rhs=ets[kc][:],
                                start=(kc == 0), stop=(kc == last),
                            )
                        prs = PSR.tile([128, 512], F32, tag="r", name="pr")
                        for kc in kcs:
                            lo = max(0, 64 * kc - half * 512)
                            nc.tensor.matmul(
                                prs[:, lo:512],
                                lhsT=onesb[:],
                                rhs=ets[kc][:],
                                start=(kc == 0), stop=(kc == last),
                            )
                        rc = RC.tile([128, 512], F32, tag="rc", name="rc")
                        nc.vector.reciprocal_approx_fast(out=rc[:], in_=prs[:])
                        nc.vector.tensor_mul(ctxT[:, h, qsl], pctx[:], rc[:])

                    # software-pipelined emission: weave next group's builds
                    # between score half-batches so the tensor engine has
                    # filler while exp drains the score ring
                    build_kT(0)
                    build_v_half(0, 0)
                    build_v_half(0, 1)
                    q_proj(0)
                    for g in range(NG):
                        if g == NG - 1:
                            for oc in range(4):
                                nc.sync.dma_start(
                                    wo_s[:, :, oc * 512:(oc + 1) * 512],
                                    wo[:, :, oc * 512:(oc + 1) * 512],
                                )
                        if g + 1 < NG:
                            build_kT_start(g + 1)
                        e00 = attn_scores(g, 0, 0, range(8))
                        if g + 1 < NG:
                            build_kT_head(g + 1, 0)
                        e01 = attn_scores(g, 0, 1, range(NKC))
                        attn_ctx(g, 0, 0, e00)
                        if g + 1 < NG:
                            build_kT_head(g + 1, 1)
                        e10 = attn_scores(g, 1, 0, range(8))
                        attn_ctx(g, 0, 1, e01)
                        if g == 0:
                            build_v_half(1, 0)
                        if g == 1:
                            build_v_half(1, 1)
                        e11 = attn_scores(g, 1, 1, range(NKC))
                        if g + 1 < NG:
                            q_proj(g + 1)
                        attn_ctx(g, 1, 0, e10)
                        attn_ctx(g, 1, 1, e11)

            # ---------- Phase 2b: out-projection ----------
            with tc.tile_pool(name="st", bufs=6) as STP, \
                 tc.tile_pool(name="ps_o", bufs=8, space="PSUM") as PO:
                for tq in range(8):
                    pos = [PO.tile([128, 512], F32, tag="o", name=f"po{oc}")
                           for oc in range(4)]
                    for h in range(HL):
                        for oc in range(4):
                            nc.tensor.matmul(
                                pos[oc][:],
                                lhsT=ctxT[:, h, tq * 128:(tq + 1) * 128],
                                rhs=wo_s[:, h, oc * 512:(oc + 1) * 512],
                                start=(h == 0), stop=(h == HL - 1),
                            )
                    for oc in range(4):
                        osl = slice(oc * 512, (oc + 1) * 512)
                        st = STP.tile([128, 512], F32, tag="st", name="st")
                        if oc % 2 == 0:
                            nc.scalar.copy(st[:], pos[oc][:])
                        else:
                            nc.vector.tensor_copy(st[:], pos[oc][:])
                        deng = (nc.sync, nc.gpsimd, nc.scalar)[oc % 3]
                        deng.dma_start(
                            out_d[tq * 128:(tq + 1) * 128, osl], st[:]
                        )

    nc.compile()
    return nc


def _rot_rows(w):
    # rows of w are the rope dim; rot(w) @ lat == rotate_half(w @ lat)
    hR = w.shape[0] // 2
    return np.concatenate([-w[hR:], w[:hR]], axis=0)


def _hidsplit(m):
    """[HID, cols] -> [128, 8, 2, cols] with hid row (2*hch+two)*128+p."""
    cols = m.shape[1]
    return np.ascontiguousarray(
        m.reshape(16, 128, cols).transpose(1, 0, 2).reshape(128, 8, 2, cols)
    )


def _latsplit(m):
    """[LAT, cols] -> [128, 4, cols] with lat row lc*128+p."""
    cols = m.shape[1]
    return np.ascontiguousarray(
        m.reshape(4, 128, cols).transpose(1, 0, 2)
    )


def _prep_inputs(inputs):
    x = np.asarray(inputs["x"], np.float32)
    Wq_down = np.asarray(inputs["Wq_down"], np.float32)
    Wq_up = np.asarray(inputs["Wq_up"], np.float32)
    Wq_rope = np.asarray(inputs["Wq_rope"], np.float32)
    Wkv_down = np.asarray(inputs["Wkv_down"], np.float32)
    Wk_up = np.asarray(inputs["Wk_up"], np.float32)
    Wk_rope = np.asarray(inputs["Wk_rope"], np.float32)
    Wv_up = np.asarray(inputs["Wv_up"], np.float32)
    Wo = np.asarray(inputs["Wo"], np.float32)

    s = np.float32(1.0 / np.sqrt(DH))

    wdkv_h = _hidsplit(Wkv_down.T).astype(BF16NP)
    wdq_h = _hidsplit(Wq_down.T).astype(BF16NP)
    wkr2_h = _hidsplit(
        np.concatenate([Wk_rope.T, _rot_rows(Wk_rope).T], axis=1)
    ).astype(BF16NP)
    wk_full = _latsplit(Wk_up.T).reshape(128, 4, H, C)
    wv_full = _latsplit(Wv_up.T).reshape(128, 4, H, DH)
    wqc_full = _latsplit((Wq_up * s).T).reshape(128, 4, H, C)
    wo_full = np.ascontiguousarray(
        Wo.T.reshape(H, 128, HID).transpose(1, 0, 2)
    )

    inv_freq = (1.0 / (10000.0 ** (np.arange(0, R, 2, dtype=np.float32) / R)))
    t = np.arange(S, dtype=np.float32)
    freqs = t[:, None] * inv_freq[None, :]
    emb = np.concatenate([freqs, freqs], axis=-1)          # [S, R]
    cos = np.cos(emb).astype(np.float32)
    sin = np.sin(emb).astype(np.float32)

    cs_k_h = np.ascontiguousarray(
        np.concatenate([cos.T, sin.T], axis=0)).astype(BF16NP)
    xbT_h = [_hidsplit(x[b].T.astype(np.float32)).astype(BF16NP)
             for b in range(B)]

    # per-jh-sliced head weights
    wk_h, wv_h, wqc_h, wqr_h, wo_h = {}, {}, {}, {}, {}
    for jh in range(2):
        hs = slice(8 * jh, 8 * jh + 8)
        wk_h[jh] = np.ascontiguousarray(wk_full[:, :, hs, :]).astype(BF16NP)
        wv_h[jh] = np.ascontiguousarray(wv_full[:, :, hs, :]).astype(BF16NP)
        wqc_h[jh] = np.ascontiguousarray(
            wqc_full[:, :, hs, :].reshape(128, 4, NG, GH, C)
            .transpose(0, 2, 1, 3, 4)
        ).astype(BF16NP)
        # per group: [ro_h0 | ro_h1 | rot_h0 | rot_h1] * s
        wqr4 = np.empty((LAT, NG, 4 * R), np.float32)
        for g in range(NG):
            h0 = 8 * jh + GH * g
            q0 = Wq_rope[h0 * R:(h0 + 1) * R] * s
            q1 = Wq_rope[(h0 + 1) * R:(h0 + 2) * R] * s
            wqr4[:, g, 0 * R:1 * R] = q0.T
            wqr4[:, g, 1 * R:2 * R] = q1.T
            wqr4[:, g, 2 * R:3 * R] = _rot_rows(q0).T
            wqr4[:, g, 3 * R:4 * R] = _rot_rows(q1).T
        wqr_h[jh] = np.ascontiguousarray(
            _latsplit(wqr4.reshape(LAT, NG * 4 * R))
            .reshape(128, 4, NG, 4 * R).transpose(0, 2, 1, 3)
        ).astype(BF16NP)
        wo_h[jh] = np.ascontiguousarray(wo_full[:, hs, :]).astype(BF16NP)

    par = np.arange(128)[:, None]
    mar = np.arange(64)[None, :]

    in_maps = []
    for c in range(8):
        b, sub = divmod(c, 4)
        jh, jq = divmod(sub, 2)
        qi = jq + 2 * np.arange(SQ)
        cs4q = np.empty((128, SQ), np.float32)
        cs4q[0:R] = cos[qi].T
        cs4q[R:2 * R] = cos[qi].T
        cs4q[2 * R:3 * R] = sin[qi].T
        cs4q[3 * R:4 * R] = sin[qi].T
        maskb = np.where(par <= 2 * mar + jq, 1.0, 0.0)
        in_maps.append({
            "xbT": xbT_h[b],
            "xqT": _hidsplit(x[b, jq::2].T.astype(np.float32)).astype(BF16NP),
            "wdkv": wdkv_h, "wdq": wdq_h, "wkr2": wkr2_h,
            "wk": wk_h[jh], "wv": wv_h[jh], "wqc": wqc_h[jh],
            "wqr": wqr_h[jh], "wo": wo_h[jh],
            "cs4q": np.ascontiguousarray(cs4q),
            "cs_k": cs_k_h,
            "maskb": maskb.astype(BF16NP),
        })
    return in_maps


_NC_CACHE = None


def run_on_cores(inputs, trace=False):
    global _NC_CACHE
    if _NC_CACHE is None:
        _NC_CACHE = build_nc()
    nc = _NC_CACHE
    in_maps = _prep_inputs(inputs)
    res = run_bass_kernel_spmd(nc, in_maps, list(range(8)), trace=trace)
    out = np.zeros((B, S, HID), np.float32)
    for c in range(8):
        b, sub = divmod(c, 4)
        jh, jq = divmod(sub, 2)
        out[b, jq::2, :] += res.results[c]["out"]
    out += np.asarray(inputs["bo"], np.float32)[None, None, :]
    return out, res


def kernel(**inputs):
    out, _ = run_on_cores(inputs, trace=False)
    return out
